# revision 9
# baseline (speedup 1.0000x reference)
"""AttForwardTA (location-aware attention + transition agent) on 8 TRN2 NeuronCores.

Data-parallel: batch 64 is split 8 ways; each core runs the identical program on
its 8 batch rows.  The big matmuls (pre-projection of enc, context) run in
float32r (TF32-like, 1 PE cycle/row); the small conv/decoder/transition-agent
matmuls run in bf16; softmax / prior / renormalization stay in exact f32.
Batches are processed in two groups of four so softmax DVE work overlaps the
next group's PE work and the encoder stash fits SBUF.
"""
import sys

sys.path.insert(0, "/opt/trn_rl_repo")
import numpy as np
import ml_dtypes
from contextlib import ExitStack

import concourse.bass as bass
import concourse.tile as tile
from concourse import mybir, bacc
from concourse.bass_utils import run_bass_kernel_spmd

F32 = mybir.dt.float32
F32R = mybir.dt.float32r
BF16 = mybir.dt.bfloat16
AF = mybir.ActivationFunctionType
ALU = mybir.AluOpType

NB = 8          # batches per core
GRP = 2         # batch groups
GB = NB // GRP  # batches per group
NCORES = 8
T = 1000
E = 512
D = 1024
A = 128
CH = 32
KW = 31
ODIM = 80
TT = 8          # T tiles of 128 (last is 104)
TLEN = [128] * 7 + [104]
EC = E // 128
DC = D // 128
HALVES = [(0, 512), (512, 488)]   # psum-bank-aligned split of T


def _emit(nc):
    enc_h = nc.dram_tensor("enc", [NB, T, E], F32R, kind="ExternalInput")
    ap_h = nc.dram_tensor("att_prev", [NB, T], F32, kind="ExternalInput")
    dz_h = nc.dram_tensor("dec_z", [NB, D], F32, kind="ExternalInput")
    op_h = nc.dram_tensor("out_prev", [NB, ODIM], F32, kind="ExternalInput")
    wenc_h = nc.dram_tensor("w_enc", [E, A], F32R, kind="ExternalInput")
    wdec_h = nc.dram_tensor("w_dec", [D, A], F32, kind="ExternalInput")
    watt_h = nc.dram_tensor("w_att", [CH, A], F32, kind="ExternalInput")
    conv_h = nc.dram_tensor("conv2", [CH, KW], F32, kind="ExternalInput")
    wg_h = nc.dram_tensor("wg", [A, 1], F32, kind="ExternalInput")
    benc_h = nc.dram_tensor("benc", [A, 1], F32, kind="ExternalInput")
    wta_h = nc.dram_tensor("wta", [13 * 128, 1], F32, kind="ExternalInput")
    bta_h = nc.dram_tensor("bta", [NB, 1], F32, kind="ExternalInput")
    len_h = nc.dram_tensor("lens", [NB, 1], F32, kind="ExternalInput")
    iota_h = nc.dram_tensor("iota", [1, T], F32, kind="ExternalInput")
    idb_h = nc.dram_tensor("identb", [128, 128], BF16, kind="ExternalInput")
    idr_h = nc.dram_tensor("identr", [128, 128], F32R, kind="ExternalInput")

    c_out = nc.dram_tensor("c_out", [NB, E], F32, kind="ExternalOutput")
    w_out = nc.dram_tensor("w_out", [NB, T], F32, kind="ExternalOutput")
    p_out = nc.dram_tensor("p_out", [NB, 1], F32, kind="ExternalOutput")

    with tile.TileContext(nc) as tc:
        with ExitStack() as ctx:
            const = ctx.enter_context(tc.tile_pool(name="const", bufs=1))
            ph2 = ctx.enter_context(tc.tile_pool(name="ph2", bufs=1))
            encp = ctx.enter_context(tc.tile_pool(name="encp", bufs=6))
            encTp = ctx.enter_context(tc.tile_pool(name="encTp", bufs=4))
            wrk = ctx.enter_context(tc.tile_pool(name="wrk", bufs=2))
            ppt = ctx.enter_context(tc.tile_pool(name="ppt", bufs=2, space="PSUM"))
            ppx = ctx.enter_context(tc.tile_pool(name="ppx", bufs=2, space="PSUM"))
            pps = ctx.enter_context(tc.tile_pool(name="pps", bufs=2, space="PSUM"))

            def cload(name, shape, dt, src, eng=None):
                t = const.tile(shape, dt, tag=name)
                (eng or nc.gpsimd).dma_start(out=t[:], in_=src)
                return t

            # ---- constants / small inputs ----
            identb = cload("identb", [128, 128], BF16, idb_h.ap(), nc.sync)
            identr = cload("identr", [128, 128], F32R, idr_h.ap(), nc.sync)
            # w_enc chunks side by side: wenc_sb[p, k*128+a] = w_enc[k*128+p, a]
            wenc_sb = const.tile([128, E], F32R, tag="wenc")
            nc.sync.dma_start(
                out=wenc_sb[:],
                in_=bass.AP(tensor=wenc_h.ap().tensor, offset=0,
                            ap=[[A, 128], [128 * A, EC], [1, A]]),
            )
            wdec_sb = const.tile([128, D], BF16, tag="wdec")
            nc.gpsimd.dma_start(
                out=wdec_sb[:],
                in_=bass.AP(tensor=wdec_h.ap().tensor, offset=0,
                            ap=[[A, 128], [128 * A, DC], [1, A]]),
            )
            watt_sb = cload("watt", [CH, A], BF16, watt_h.ap())
            conv_sb = cload("conv", [CH, KW], BF16, conv_h.ap())
            wg_sb = cload("wg", [A, 1], BF16, wg_h.ap())
            benc_sb = cload("benc", [A, 1], F32, benc_h.ap(), nc.sync)
            wta_sb = const.tile([128, 13], BF16, tag="wta")
            nc.gpsimd.dma_start(
                out=wta_sb[:],
                in_=bass.AP(tensor=wta_h.ap().tensor, offset=0,
                            ap=[[1, 128], [128, 13]]),
            )
            bta_sb = cload("bta", [NB, 1], F32, bta_h.ap(), nc.sync)
            dzbf = cload("dzbf", [NB, D], BF16, dz_h.ap())
            opbf = cload("opbf", [NB, ODIM], BF16, op_h.ap())
            # padded att_prev (bf16) for the location conv im2col
            pad_sb = const.tile([NB, T + 30], BF16, tag="pad")
            nc.vector.memset(pad_sb[:], 0.0)
            nc.gpsimd.dma_start(out=pad_sb[:, 15:15 + T], in_=ap_h.ap())

            iota_sb = const.tile([GB, T], F32, tag="iota")
            nc.sync.dma_start(
                out=iota_sb[:],
                in_=bass.AP(tensor=iota_h.ap().tensor, offset=0, ap=[[0, GB], [1, T]]),
            )
            ap32_g = []
            negm_g = []
            for g in range(GRP):
                apg = const.tile([GB, T], F32, tag=f"ap32_{g}")
                nc.sync.dma_start(out=apg[:], in_=ap_h.ap()[g * GB:(g + 1) * GB, :])
                ap32_g.append(apg)
                leng = const.tile([GB, 1], F32, tag=f"len_{g}")
                nc.sync.dma_start(out=leng[:], in_=len_h.ap()[g * GB:(g + 1) * GB, :])
                nm = const.tile([GB, T], F32, tag=f"negm_{g}")
                nc.vector.tensor_scalar(
                    out=nm[:], in0=iota_sb[:],
                    scalar1=leng[:, 0:1], scalar2=-1.0e30,
                    op0=ALU.is_ge, op1=ALU.mult,
                )
                negm_g.append(nm)

            # ---- dec_z^T (reused by the transition-agent tail) and dec_t ----
            dzT = const.tile([128, DC * NB], BF16, tag="dzT")
            for d in range(DC):
                tr = pps.tile([128, NB], BF16, tag="ps")
                nc.tensor.transpose(
                    out=tr[:], in_=dzbf[:, d * 128:(d + 1) * 128],
                    identity=identb[0:NB, 0:NB],
                )
                nc.vector.tensor_copy(out=dzT[:, d * NB:(d + 1) * NB], in_=tr[:])
            pdec = pps.tile([128, NB], F32, tag="ps")
            for d in range(DC):
                nc.tensor.matmul(
                    pdec[:], wdec_sb[:, d * 128:(d + 1) * 128],
                    dzT[:, d * NB:(d + 1) * NB],
                    start=(d == 0), stop=(d == DC - 1),
                )
            bias_sb = const.tile([128, NB], F32, tag="bias")
            nc.vector.tensor_scalar_add(bias_sb[:], pdec[:], benc_sb[:, 0:1])

            # ---- folded conv weights: g[k, a] = sum_c conv[c, k] * w_att[c, a] ----
            pg = pps.tile([KW, A], F32, tag="ps")
            nc.tensor.matmul(pg[:], conv_sb[:], watt_sb[:], start=True, stop=True)
            g_sb = const.tile([KW, A], BF16, tag="g")
            nc.vector.tensor_copy(out=g_sb[:], in_=pg[:])

            c_all = const.tile([NB, E], F32, tag="c_all")
            enc_tiles = [None] * NB

            for g in range(GRP):
                bs = range(g * GB, (g + 1) * GB)
                # ============ phase 1: scores for this group's batches ========
                e_all = ph2.tile([GB, T], F32, tag="e_all")
                for b in bs:
                    enc_b = encp.tile([128, TT * E], F32R, tag="enc")
                    enc_tiles[b] = enc_b
                    nc.sync.dma_start(
                        out=enc_b[:, 0:7 * E],
                        in_=enc_h.ap()[b, 0:896, :].rearrange(
                            "(t p) e -> p t e", p=128),
                    )
                    nc.sync.dma_start(
                        out=enc_b[0:104, 7 * E:8 * E],
                        in_=enc_h.ap()[b, 896:T, :],
                    )

                    encTs = []
                    for k in range(EC):
                        ek = encTp.tile([128, T], F32R, tag="encT")
                        for hi, (h0, hw_) in enumerate(HALVES):
                            ptk = ppt.tile([128, 512], F32R, tag="pt")
                            for t in range(hi * 4, hi * 4 + 4):
                                tl = TLEN[t]
                                nc.tensor.transpose(
                                    out=ptk[:, (t - hi * 4) * 128:
                                            (t - hi * 4) * 128 + tl],
                                    in_=enc_b[0:tl,
                                              t * E + k * 128:t * E + (k + 1) * 128],
                                    identity=identr[0:tl, 0:tl],
                                )
                            if (k + hi) % 2 == 0:
                                nc.vector.tensor_copy(
                                    out=ek[:, h0:h0 + hw_], in_=ptk[:, 0:hw_])
                            else:
                                nc.scalar.copy(
                                    out=ek[:, h0:h0 + hw_], in_=ptk[:, 0:hw_])
                        encTs.append(ek)

                    X_b = wrk.tile([KW, T], BF16, tag="X")
                    pad_ap = pad_sb[:]
                    nc.gpsimd.dma_start(
                        out=X_b[:],
                        in_=bass.AP(
                            tensor=pad_ap.tensor,
                            offset=pad_ap.offset + b * (T + 30),
                            ap=[[T + 30, 1], [1, KW], [1, T]],
                        ),
                    )

                    px = ppx.tile([128, T], F32, tag="px")
                    for h0, hw_ in HALVES:
                        for k in range(EC):
                            nc.tensor.matmul(
                                px[:, h0:h0 + hw_],
                                wenc_sb[:, k * 128:(k + 1) * 128],
                                encTs[k][:, h0:h0 + hw_],
                                start=(k == 0), stop=False,
                            )
                        nc.tensor.matmul(
                            px[:, h0:h0 + hw_], g_sb[:], X_b[:, h0:h0 + hw_],
                            start=False, stop=True,
                        )

                    tanh_b = wrk.tile([128, T], BF16, tag="tanh")
                    nc.scalar.activation(
                        tanh_b[:], px[:], AF.Tanh, bias=bias_sb[:, b:b + 1],
                        scale=1.0,
                    )

                    erow = wrk.tile([1, T], F32, tag="erow")
                    for hi, (h0, hw_) in enumerate(HALVES):
                        pe = pps.tile([1, 512], F32, tag="ps")
                        nc.tensor.matmul(
                            pe[0:1, 0:hw_], wg_sb[:], tanh_b[:, h0:h0 + hw_],
                            start=True, stop=True,
                        )
                        if (b + hi) % 2 == 0:
                            nc.vector.tensor_copy(
                                out=erow[0:1, h0:h0 + hw_], in_=pe[0:1, 0:hw_])
                        else:
                            nc.scalar.copy(
                                out=erow[0:1, h0:h0 + hw_], in_=pe[0:1, 0:hw_])
                    nc.sync.dma_start(
                        out=e_all[b - g * GB:b - g * GB + 1, :], in_=erow[:])

                # ============ phase 2: softmax + prior + renormalize ==========
                e_m = ph2.tile([GB, T], F32, tag="e_m")
                nc.vector.tensor_add(e_m[:], e_all[:], negm_g[g][:])
                rmax = ph2.tile([GB, 1], F32, tag="rmax")
                nc.vector.tensor_reduce(
                    out=rmax[:], in_=e_m[:], axis=mybir.AxisListType.X, op=ALU.max)
                rneg = ph2.tile([GB, 1], F32, tag="rneg")
                nc.vector.tensor_scalar_mul(rneg[:], rmax[:], -1.0)
                ex_t = ph2.tile([GB, T], F32, tag="ex_t")
                ssum = ph2.tile([GB, 1], F32, tag="ssum")
                nc.scalar.activation(
                    ex_t[:], e_m[:], AF.Exp, bias=rneg[:, 0:1], scale=1.0,
                    accum_out=ssum[:],
                )
                thr = ph2.tile([GB, 1], F32, tag="thr")
                nc.vector.tensor_scalar_mul(thr[:], ssum[:], 2.0e-6)
                padd = ph2.tile([GB, T], F32, tag="padd")
                nc.vector.tensor_add(
                    padd[:, 1:T], ap32_g[g][:, 1:T], ap32_g[g][:, 0:T - 1])
                nc.vector.tensor_copy(out=padd[:, 0:1], in_=ap32_g[g][:, 0:1])
                q_t = ph2.tile([GB, T], F32, tag="q_t")
                nc.vector.tensor_mul(q_t[:], padd[:], ex_t[:])
                qc = ph2.tile([GB, T], F32, tag="qc")
                qsum = ph2.tile([GB, 1], F32, tag="qsum")
                nc.vector.tensor_scalar(
                    out=qc[:], in0=q_t[:], scalar1=thr[:, 0:1], scalar2=None,
                    op0=ALU.max, op1=ALU.add, accum_out=qsum[:],
                )
                winv = ph2.tile([GB, 1], F32, tag="winv")
                nc.vector.reciprocal(winv[:], qsum[:])
                w32 = ph2.tile([GB, T], F32, tag="w32")
                nc.vector.tensor_scalar_mul(w32[:], qc[:], winv[:, 0:1])
                nc.sync.dma_start(
                    out=w_out.ap()[g * GB:(g + 1) * GB, :], in_=w32[:])

                wr = ph2.tile([GB, T], F32R, tag="wr")
                nc.vector.tensor_copy(out=wr[:], in_=w32[:])
                wT_sb = ph2.tile([128, TT * GB], F32R, tag="wT")
                for j in range(TT):
                    tl = TLEN[j]
                    tr = pps.tile([128, GB], F32R, tag="ps")
                    nc.tensor.transpose(
                        out=tr[0:tl, :], in_=wr[:, j * 128:j * 128 + tl],
                        identity=identr[0:GB, 0:GB],
                    )
                    nc.vector.tensor_copy(
                        out=wT_sb[0:tl, j * GB:(j + 1) * GB], in_=tr[0:tl, :])

                # ============ phase 3: context ============
                for b in bs:
                    pc = pps.tile([1, 512], F32, tag="ps")
                    for j in range(TT):
                        tl = TLEN[j]
                        nc.tensor.matmul(
                            pc[:],
                            wT_sb[0:tl, j * GB + (b - g * GB):
                                  j * GB + (b - g * GB) + 1],
                            enc_tiles[b][0:tl, j * E:(j + 1) * E],
                            start=(j == 0), stop=(j == TT - 1),
                        )
                    crow = wrk.tile([1, E], F32, tag="crow")
                    if b % 2 == 0:
                        nc.vector.tensor_copy(out=crow[:], in_=pc[:])
                    else:
                        nc.scalar.copy(out=crow[:], in_=pc[:])
                    nc.sync.dma_start(out=c_all[b:b + 1, :], in_=crow[:])

            nc.sync.dma_start(out=c_out.ap(), in_=c_all[:])

            # ================= phase 4: transition agent prob =================
            cbf = const.tile([NB, E], BF16, tag="cbf")
            nc.vector.tensor_copy(out=cbf[:], in_=c_all[:])
            cT = const.tile([128, 4 * NB], BF16, tag="cT")
            for i in range(4):
                tr = pps.tile([128, NB], BF16, tag="ps")
                nc.tensor.transpose(
                    out=tr[:], in_=cbf[:, i * 128:(i + 1) * 128],
                    identity=identb[0:NB, 0:NB],
                )
                nc.vector.tensor_copy(out=cT[:, i * NB:(i + 1) * NB], in_=tr[:])
            opT = const.tile([128, NB], BF16, tag="opT")
            trop = pps.tile([128, NB], BF16, tag="ps")
            nc.tensor.transpose(
                out=trop[0:ODIM, :], in_=opbf[:, 0:ODIM],
                identity=identb[0:NB, 0:NB],
            )
            nc.vector.tensor_copy(out=opT[0:ODIM, :], in_=trop[0:ODIM, :])

            ps_s = pps.tile([NB, 1], F32, tag="ps")
            for i in range(13):
                if i < 4:
                    lhsT = cT[:, i * NB:(i + 1) * NB]
                    rhs = wta_sb[:, i:i + 1]
                elif i < 12:
                    lhsT = dzT[:, (i - 4) * NB:(i - 3) * NB]
                    rhs = wta_sb[:, i:i + 1]
                else:
                    lhsT = opT[0:ODIM, :]
                    rhs = wta_sb[0:ODIM, i:i + 1]
                nc.tensor.matmul(ps_s[:], lhsT, rhs, start=(i == 0), stop=(i == 12))
            s_sb = const.tile([NB, 1], F32, tag="s_sb")
            nc.vector.tensor_scalar_add(s_sb[:], ps_s[:], bta_sb[:, 0:1])
            exs = const.tile([NB, 1], F32, tag="exs")
            nc.scalar.activation(exs[:], s_sb[:], AF.Exp, bias=0.0, scale=-1.0)
            p1 = const.tile([NB, 1], F32, tag="p1")
            nc.vector.tensor_scalar_add(p1[:], exs[:], 1.0)
            prob = const.tile([NB, 1], F32, tag="prob")
            nc.vector.reciprocal(prob[:], p1[:])
            nc.sync.dma_start(out=p_out.ap(), in_=prob[:])

    return nc


_NC = None


def _get_nc():
    global _NC
    if _NC is None:
        nc = bacc.Bacc("TRN2", target_bir_lowering=False, debug=False,
                       num_devices=NCORES)
        _emit(nc)
        nc.compile()
        _NC = nc
    return _NC


def make_in_maps(inputs):
    f32 = np.float32
    enc = np.asarray(inputs["enc_hs_pad"], f32)
    lens = np.asarray(inputs["enc_hs_len"], np.int32)
    dec_z = np.asarray(inputs["dec_z"], f32)
    att_prev = np.asarray(inputs["att_prev"], f32)
    out_prev = np.asarray(inputs["out_prev"], f32)
    w_enc = np.asarray(inputs["w_enc"], f32)
    w_dec = np.asarray(inputs["w_dec"], f32)
    w_att = np.asarray(inputs["w_att"], f32)
    conv_w = np.asarray(inputs["conv_w"], f32)
    w_g = np.asarray(inputs["w_g"], f32)
    w_ta = np.asarray(inputs["w_ta"], f32)
    b_ta = np.asarray(inputs["b_ta"], f32)
    b_enc = np.asarray(inputs["b_enc"], f32)

    conv2 = np.ascontiguousarray(conv_w[:, 0, :])                  # [32, 31]
    wg_col = np.ascontiguousarray(w_g.reshape(A, 1))
    benc_col = np.ascontiguousarray(b_enc.reshape(A, 1))
    # ta input order on device: [c | dec_z | out_prev]; pad to 13*128 rows
    wta_r = np.zeros((13 * 128, 1), f32)
    wta_r[0:E, 0] = w_ta[0:E, 0]
    wta_r[E:E + D, 0] = w_ta[E + ODIM:E + ODIM + D, 0]
    wta_r[E + D:E + D + ODIM, 0] = w_ta[E:E + ODIM, 0]
    iota = np.arange(T, dtype=f32).reshape(1, T)
    identb = np.eye(128, dtype=f32).astype(ml_dtypes.bfloat16)
    identr = np.eye(128, dtype=f32)

    in_maps = []
    for i in range(NCORES):
        sl = slice(i * NB, (i + 1) * NB)
        in_maps.append({
            "enc": np.ascontiguousarray(enc[sl]),
            "att_prev": np.ascontiguousarray(att_prev[sl]),
            "dec_z": np.ascontiguousarray(dec_z[sl]),
            "out_prev": np.ascontiguousarray(out_prev[sl]),
            "w_enc": w_enc,
            "w_dec": w_dec,
            "w_att": w_att,
            "conv2": conv2,
            "wg": wg_col,
            "benc": benc_col,
            "wta": wta_r,
            "bta": np.full((NB, 1), float(b_ta[0]), f32),
            "lens": lens[sl].astype(f32).reshape(NB, 1),
            "iota": iota,
            "identb": identb,
            "identr": identr,
        })
    return in_maps


def kernel(**inputs):
    nc = _get_nc()
    in_maps = make_in_maps(inputs)
    res = run_bass_kernel_spmd(nc, in_maps, list(range(NCORES)))
    c = np.concatenate([res.results[i]["c_out"] for i in range(NCORES)], axis=0)
    w = np.concatenate([res.results[i]["w_out"] for i in range(NCORES)], axis=0)
    p = np.concatenate([res.results[i]["p_out"] for i in range(NCORES)], axis=0)
    return c, w, p


# revision 11
# speedup vs baseline: 1.1794x; 1.1794x over previous
"""AttForwardTA (location-aware attention + transition agent) on 8 TRN2 NeuronCores.

Data-parallel: batch 64 is split 8 ways; each core runs the identical program on
its 8 batch rows.  The big matmuls (pre-projection of enc, context) run in
float32r (TF32-like, 1 PE cycle/row); the small conv/decoder/transition-agent
matmuls run in bf16; softmax / prior / renormalization stay in exact f32.
Batches are processed in two groups of four so softmax DVE work overlaps the
next group's PE work and the encoder stash fits SBUF.
"""
import sys

sys.path.insert(0, "/opt/trn_rl_repo")
import numpy as np
import ml_dtypes
from contextlib import ExitStack

import concourse.bass as bass
import concourse.tile as tile
from concourse import mybir, bacc
from concourse.bass_utils import run_bass_kernel_spmd

F32 = mybir.dt.float32
F32R = mybir.dt.float32r
BF16 = mybir.dt.bfloat16
AF = mybir.ActivationFunctionType
ALU = mybir.AluOpType

NB = 8          # batches per core
GRP = 2         # batch groups
GB = NB // GRP  # batches per group
NCORES = 8
T = 1000
E = 512
D = 1024
A = 128
CH = 32
KW = 31
ODIM = 80
TT = 8          # T tiles of 128 (last is 104)
TLEN = [128] * 7 + [104]
EC = E // 128
DC = D // 128
HALVES = [(0, 512), (512, 488)]   # psum-bank-aligned split of T


def _emit(nc):
    enc_h = nc.dram_tensor("enc", [NB, T, E], F32, kind="ExternalInput")
    ap_h = nc.dram_tensor("att_prev", [NB, T], F32, kind="ExternalInput")
    dz_h = nc.dram_tensor("dec_z", [NB, D], F32, kind="ExternalInput")
    op_h = nc.dram_tensor("out_prev", [NB, ODIM], F32, kind="ExternalInput")
    wenc_h = nc.dram_tensor("w_enc", [E, A], F32, kind="ExternalInput")
    wdec_h = nc.dram_tensor("w_dec", [D, A], F32, kind="ExternalInput")
    watt_h = nc.dram_tensor("w_att", [CH, A], F32, kind="ExternalInput")
    conv_h = nc.dram_tensor("conv2", [CH, KW], F32, kind="ExternalInput")
    wg_h = nc.dram_tensor("wg", [A, 1], F32, kind="ExternalInput")
    benc_h = nc.dram_tensor("benc", [A, 1], F32, kind="ExternalInput")
    wta_h = nc.dram_tensor("wta", [13 * 128, 1], F32, kind="ExternalInput")
    bta_h = nc.dram_tensor("bta", [NB, 1], F32, kind="ExternalInput")
    len_h = nc.dram_tensor("lens", [NB, 1], F32, kind="ExternalInput")
    iota_h = nc.dram_tensor("iota", [1, T], F32, kind="ExternalInput")
    idb_h = nc.dram_tensor("identb", [128, 128], BF16, kind="ExternalInput")

    c_out = nc.dram_tensor("c_out", [NB, E], F32, kind="ExternalOutput")
    w_out = nc.dram_tensor("w_out", [NB, T], F32, kind="ExternalOutput")
    p_out = nc.dram_tensor("p_out", [NB, 1], F32, kind="ExternalOutput")

    with tile.TileContext(nc) as tc:
        with ExitStack() as ctx:
            const = ctx.enter_context(tc.tile_pool(name="const", bufs=1))
            ph2 = ctx.enter_context(tc.tile_pool(name="ph2", bufs=1))
            encp = ctx.enter_context(tc.tile_pool(name="encp", bufs=6))
            encTp = ctx.enter_context(tc.tile_pool(name="encTp", bufs=4))
            wrk = ctx.enter_context(tc.tile_pool(name="wrk", bufs=2))
            ppt = ctx.enter_context(tc.tile_pool(name="ppt", bufs=2, space="PSUM"))
            ppx = ctx.enter_context(tc.tile_pool(name="ppx", bufs=2, space="PSUM"))
            pps = ctx.enter_context(tc.tile_pool(name="pps", bufs=2, space="PSUM"))

            def cload(name, shape, dt, src, eng=None):
                t = const.tile(shape, dt, tag=name)
                (eng or nc.gpsimd).dma_start(out=t[:], in_=src)
                return t

            # ---- constants / small inputs ----
            identb = cload("identb", [128, 128], BF16, idb_h.ap(), nc.sync)
            # w_enc chunks side by side: wenc_sb[p, k*128+a] = w_enc[k*128+p, a]
            wenc_sb = const.tile([128, E], BF16, tag="wenc")
            nc.gpsimd.dma_start(
                out=wenc_sb[:],
                in_=bass.AP(tensor=wenc_h.ap().tensor, offset=0,
                            ap=[[A, 128], [128 * A, EC], [1, A]]),
            )
            wdec_sb = const.tile([128, D], BF16, tag="wdec")
            nc.gpsimd.dma_start(
                out=wdec_sb[:],
                in_=bass.AP(tensor=wdec_h.ap().tensor, offset=0,
                            ap=[[A, 128], [128 * A, DC], [1, A]]),
            )
            watt_sb = cload("watt", [CH, A], BF16, watt_h.ap())
            conv_sb = cload("conv", [CH, KW], BF16, conv_h.ap())
            wg_sb = cload("wg", [A, 1], BF16, wg_h.ap())
            benc_sb = cload("benc", [A, 1], F32, benc_h.ap(), nc.sync)
            wta_sb = const.tile([128, 13], BF16, tag="wta")
            nc.gpsimd.dma_start(
                out=wta_sb[:],
                in_=bass.AP(tensor=wta_h.ap().tensor, offset=0,
                            ap=[[1, 128], [128, 13]]),
            )
            bta_sb = cload("bta", [NB, 1], F32, bta_h.ap(), nc.sync)
            dzbf = cload("dzbf", [NB, D], BF16, dz_h.ap())
            opbf = cload("opbf", [NB, ODIM], BF16, op_h.ap())
            # padded att_prev (bf16) for the location conv im2col
            pad_sb = const.tile([NB, T + 30], BF16, tag="pad")
            nc.vector.memset(pad_sb[:], 0.0)
            nc.gpsimd.dma_start(out=pad_sb[:, 15:15 + T], in_=ap_h.ap())

            iota_sb = const.tile([GB, T], F32, tag="iota")
            nc.sync.dma_start(
                out=iota_sb[:],
                in_=bass.AP(tensor=iota_h.ap().tensor, offset=0, ap=[[0, GB], [1, T]]),
            )
            ap32_g = []
            negm_g = []
            for g in range(GRP):
                apg = const.tile([GB, T], F32, tag=f"ap32_{g}")
                nc.sync.dma_start(out=apg[:], in_=ap_h.ap()[g * GB:(g + 1) * GB, :])
                ap32_g.append(apg)
                leng = const.tile([GB, 1], F32, tag=f"len_{g}")
                nc.sync.dma_start(out=leng[:], in_=len_h.ap()[g * GB:(g + 1) * GB, :])
                nm = const.tile([GB, T], F32, tag=f"negm_{g}")
                nc.vector.tensor_scalar(
                    out=nm[:], in0=iota_sb[:],
                    scalar1=leng[:, 0:1], scalar2=-1.0e30,
                    op0=ALU.is_ge, op1=ALU.mult,
                )
                negm_g.append(nm)

            # ---- dec_z^T (reused by the transition-agent tail) and dec_t ----
            dzT = const.tile([128, DC * NB], BF16, tag="dzT")
            for d in range(DC):
                tr = pps.tile([128, NB], BF16, tag="ps")
                nc.tensor.transpose(
                    out=tr[:], in_=dzbf[:, d * 128:(d + 1) * 128],
                    identity=identb[0:NB, 0:NB],
                )
                nc.vector.tensor_copy(out=dzT[:, d * NB:(d + 1) * NB], in_=tr[:])
            pdec = pps.tile([128, NB], F32, tag="ps")
            for d in range(DC):
                nc.tensor.matmul(
                    pdec[:], wdec_sb[:, d * 128:(d + 1) * 128],
                    dzT[:, d * NB:(d + 1) * NB],
                    start=(d == 0), stop=(d == DC - 1),
                )
            bias_sb = const.tile([128, NB], F32, tag="bias")
            nc.vector.tensor_scalar_add(bias_sb[:], pdec[:], benc_sb[:, 0:1])

            # ---- folded conv weights: g[k, a] = sum_c conv[c, k] * w_att[c, a] ----
            pg = pps.tile([KW, A], F32, tag="ps")
            nc.tensor.matmul(pg[:], conv_sb[:], watt_sb[:], start=True, stop=True)
            g_sb = const.tile([KW, A], BF16, tag="g")
            nc.vector.tensor_copy(out=g_sb[:], in_=pg[:])

            c_all = const.tile([NB, E], F32, tag="c_all")
            enc_tiles = [None] * NB

            for g in range(GRP):
                bs = range(g * GB, (g + 1) * GB)
                # ============ phase 1: scores for this group's batches ========
                e_all = ph2.tile([GB, T], F32, tag="e_all")
                for b in bs:
                    enc_b = encp.tile([128, TT * E], BF16, tag="enc")
                    enc_tiles[b] = enc_b
                    nc.gpsimd.dma_start(
                        out=enc_b[:, 0:7 * E],
                        in_=enc_h.ap()[b, 0:896, :].rearrange(
                            "(t p) e -> p t e", p=128),
                    )
                    nc.gpsimd.dma_start(
                        out=enc_b[0:104, 7 * E:8 * E],
                        in_=enc_h.ap()[b, 896:T, :],
                    )

                    encTs = []
                    for k in range(EC):
                        ek = encTp.tile([128, T], BF16, tag="encT")
                        ptk = ppt.tile([128, 1024], BF16, tag="pt")
                        for t in range(TT):
                            tl = TLEN[t]
                            nc.tensor.transpose(
                                out=ptk[:, t * 128:t * 128 + tl],
                                in_=enc_b[0:tl,
                                          t * E + k * 128:t * E + (k + 1) * 128],
                                identity=identb[0:tl, 0:tl],
                            )
                        if k % 2 == 0:
                            nc.vector.tensor_copy(out=ek[:], in_=ptk[:, 0:T])
                        else:
                            nc.scalar.copy(out=ek[:], in_=ptk[:, 0:T])
                        encTs.append(ek)

                    X_b = wrk.tile([KW, T], BF16, tag="X")
                    pad_ap = pad_sb[:]
                    nc.gpsimd.dma_start(
                        out=X_b[:],
                        in_=bass.AP(
                            tensor=pad_ap.tensor,
                            offset=pad_ap.offset + b * (T + 30),
                            ap=[[T + 30, 1], [1, KW], [1, T]],
                        ),
                    )

                    px = ppx.tile([128, T], F32, tag="px")
                    for h0, hw_ in HALVES:
                        for k in range(EC):
                            nc.tensor.matmul(
                                px[:, h0:h0 + hw_],
                                wenc_sb[:, k * 128:(k + 1) * 128],
                                encTs[k][:, h0:h0 + hw_],
                                start=(k == 0), stop=False,
                            )
                        nc.tensor.matmul(
                            px[:, h0:h0 + hw_], g_sb[:], X_b[:, h0:h0 + hw_],
                            start=False, stop=True,
                        )

                    tanh_b = wrk.tile([128, T], BF16, tag="tanh")
                    nc.scalar.activation(
                        tanh_b[:], px[:], AF.Tanh, bias=bias_sb[:, b:b + 1],
                        scale=1.0,
                    )

                    erow = wrk.tile([1, T], F32, tag="erow")
                    for hi, (h0, hw_) in enumerate(HALVES):
                        pe = pps.tile([1, 512], F32, tag="ps")
                        nc.tensor.matmul(
                            pe[0:1, 0:hw_], wg_sb[:], tanh_b[:, h0:h0 + hw_],
                            start=True, stop=True,
                        )
                        if (b + hi) % 2 == 0:
                            nc.vector.tensor_copy(
                                out=erow[0:1, h0:h0 + hw_], in_=pe[0:1, 0:hw_])
                        else:
                            nc.scalar.copy(
                                out=erow[0:1, h0:h0 + hw_], in_=pe[0:1, 0:hw_])
                    nc.sync.dma_start(
                        out=e_all[b - g * GB:b - g * GB + 1, :], in_=erow[:])

                # ============ phase 2: softmax + prior + renormalize ==========
                e_m = ph2.tile([GB, T], F32, tag="e_m")
                nc.vector.tensor_add(e_m[:], e_all[:], negm_g[g][:])
                rmax = ph2.tile([GB, 1], F32, tag="rmax")
                nc.vector.tensor_reduce(
                    out=rmax[:], in_=e_m[:], axis=mybir.AxisListType.X, op=ALU.max)
                rneg = ph2.tile([GB, 1], F32, tag="rneg")
                nc.vector.tensor_scalar_mul(rneg[:], rmax[:], -1.0)
                ex_t = ph2.tile([GB, T], F32, tag="ex_t")
                ssum = ph2.tile([GB, 1], F32, tag="ssum")
                nc.scalar.activation(
                    ex_t[:], e_m[:], AF.Exp, bias=rneg[:, 0:1], scale=1.0,
                    accum_out=ssum[:],
                )
                thr = ph2.tile([GB, 1], F32, tag="thr")
                nc.vector.tensor_scalar_mul(thr[:], ssum[:], 2.0e-6)
                padd = ph2.tile([GB, T], F32, tag="padd")
                nc.vector.tensor_add(
                    padd[:, 1:T], ap32_g[g][:, 1:T], ap32_g[g][:, 0:T - 1])
                nc.vector.tensor_copy(out=padd[:, 0:1], in_=ap32_g[g][:, 0:1])
                q_t = ph2.tile([GB, T], F32, tag="q_t")
                nc.vector.tensor_mul(q_t[:], padd[:], ex_t[:])
                qc = ph2.tile([GB, T], F32, tag="qc")
                qsum = ph2.tile([GB, 1], F32, tag="qsum")
                nc.vector.tensor_scalar(
                    out=qc[:], in0=q_t[:], scalar1=thr[:, 0:1], scalar2=None,
                    op0=ALU.max, op1=ALU.add, accum_out=qsum[:],
                )
                winv = ph2.tile([GB, 1], F32, tag="winv")
                nc.vector.reciprocal(winv[:], qsum[:])
                w32 = ph2.tile([GB, T], F32, tag="w32")
                nc.vector.tensor_scalar_mul(w32[:], qc[:], winv[:, 0:1])
                nc.sync.dma_start(
                    out=w_out.ap()[g * GB:(g + 1) * GB, :], in_=w32[:])

                wr = ph2.tile([GB, T], BF16, tag="wr")
                nc.vector.tensor_copy(out=wr[:], in_=w32[:])
                wT_sb = ph2.tile([128, TT * GB], BF16, tag="wT")
                for j in range(TT):
                    tl = TLEN[j]
                    tr = pps.tile([128, GB], BF16, tag="ps")
                    nc.tensor.transpose(
                        out=tr[0:tl, :], in_=wr[:, j * 128:j * 128 + tl],
                        identity=identb[0:GB, 0:GB],
                    )
                    nc.vector.tensor_copy(
                        out=wT_sb[0:tl, j * GB:(j + 1) * GB], in_=tr[0:tl, :])

                # ============ phase 3: context ============
                for b in bs:
                    pc = pps.tile([1, 512], F32, tag="ps")
                    for j in range(TT):
                        tl = TLEN[j]
                        nc.tensor.matmul(
                            pc[:],
                            wT_sb[0:tl, j * GB + (b - g * GB):
                                  j * GB + (b - g * GB) + 1],
                            enc_tiles[b][0:tl, j * E:(j + 1) * E],
                            start=(j == 0), stop=(j == TT - 1),
                        )
                    crow = wrk.tile([1, E], F32, tag="crow")
                    if b % 2 == 0:
                        nc.vector.tensor_copy(out=crow[:], in_=pc[:])
                    else:
                        nc.scalar.copy(out=crow[:], in_=pc[:])
                    nc.sync.dma_start(out=c_all[b:b + 1, :], in_=crow[:])

            nc.sync.dma_start(out=c_out.ap(), in_=c_all[:])

            # ================= phase 4: transition agent prob =================
            cbf = const.tile([NB, E], BF16, tag="cbf")
            nc.vector.tensor_copy(out=cbf[:], in_=c_all[:])
            cT = const.tile([128, 4 * NB], BF16, tag="cT")
            for i in range(4):
                tr = pps.tile([128, NB], BF16, tag="ps")
                nc.tensor.transpose(
                    out=tr[:], in_=cbf[:, i * 128:(i + 1) * 128],
                    identity=identb[0:NB, 0:NB],
                )
                nc.vector.tensor_copy(out=cT[:, i * NB:(i + 1) * NB], in_=tr[:])
            opT = const.tile([128, NB], BF16, tag="opT")
            trop = pps.tile([128, NB], BF16, tag="ps")
            nc.tensor.transpose(
                out=trop[0:ODIM, :], in_=opbf[:, 0:ODIM],
                identity=identb[0:NB, 0:NB],
            )
            nc.vector.tensor_copy(out=opT[0:ODIM, :], in_=trop[0:ODIM, :])

            ps_s = pps.tile([NB, 1], F32, tag="ps")
            for i in range(13):
                if i < 4:
                    lhsT = cT[:, i * NB:(i + 1) * NB]
                    rhs = wta_sb[:, i:i + 1]
                elif i < 12:
                    lhsT = dzT[:, (i - 4) * NB:(i - 3) * NB]
                    rhs = wta_sb[:, i:i + 1]
                else:
                    lhsT = opT[0:ODIM, :]
                    rhs = wta_sb[0:ODIM, i:i + 1]
                nc.tensor.matmul(ps_s[:], lhsT, rhs, start=(i == 0), stop=(i == 12))
            s_sb = const.tile([NB, 1], F32, tag="s_sb")
            nc.vector.tensor_scalar_add(s_sb[:], ps_s[:], bta_sb[:, 0:1])
            exs = const.tile([NB, 1], F32, tag="exs")
            nc.scalar.activation(exs[:], s_sb[:], AF.Exp, bias=0.0, scale=-1.0)
            p1 = const.tile([NB, 1], F32, tag="p1")
            nc.vector.tensor_scalar_add(p1[:], exs[:], 1.0)
            prob = const.tile([NB, 1], F32, tag="prob")
            nc.vector.reciprocal(prob[:], p1[:])
            nc.sync.dma_start(out=p_out.ap(), in_=prob[:])

    return nc


_NC = None


def _get_nc():
    global _NC
    if _NC is None:
        nc = bacc.Bacc("TRN2", target_bir_lowering=False, debug=False,
                       num_devices=NCORES)
        _emit(nc)
        nc.compile()
        _NC = nc
    return _NC


def make_in_maps(inputs):
    f32 = np.float32
    enc = np.asarray(inputs["enc_hs_pad"], f32)
    lens = np.asarray(inputs["enc_hs_len"], np.int32)
    dec_z = np.asarray(inputs["dec_z"], f32)
    att_prev = np.asarray(inputs["att_prev"], f32)
    out_prev = np.asarray(inputs["out_prev"], f32)
    w_enc = np.asarray(inputs["w_enc"], f32)
    w_dec = np.asarray(inputs["w_dec"], f32)
    w_att = np.asarray(inputs["w_att"], f32)
    conv_w = np.asarray(inputs["conv_w"], f32)
    w_g = np.asarray(inputs["w_g"], f32)
    w_ta = np.asarray(inputs["w_ta"], f32)
    b_ta = np.asarray(inputs["b_ta"], f32)
    b_enc = np.asarray(inputs["b_enc"], f32)

    conv2 = np.ascontiguousarray(conv_w[:, 0, :])                  # [32, 31]
    wg_col = np.ascontiguousarray(w_g.reshape(A, 1))
    benc_col = np.ascontiguousarray(b_enc.reshape(A, 1))
    # ta input order on device: [c | dec_z | out_prev]; pad to 13*128 rows
    wta_r = np.zeros((13 * 128, 1), f32)
    wta_r[0:E, 0] = w_ta[0:E, 0]
    wta_r[E:E + D, 0] = w_ta[E + ODIM:E + ODIM + D, 0]
    wta_r[E + D:E + D + ODIM, 0] = w_ta[E:E + ODIM, 0]
    iota = np.arange(T, dtype=f32).reshape(1, T)
    identb = np.eye(128, dtype=f32).astype(ml_dtypes.bfloat16)

    in_maps = []
    for i in range(NCORES):
        sl = slice(i * NB, (i + 1) * NB)
        in_maps.append({
            "enc": np.ascontiguousarray(enc[sl]),
            "att_prev": np.ascontiguousarray(att_prev[sl]),
            "dec_z": np.ascontiguousarray(dec_z[sl]),
            "out_prev": np.ascontiguousarray(out_prev[sl]),
            "w_enc": w_enc,
            "w_dec": w_dec,
            "w_att": w_att,
            "conv2": conv2,
            "wg": wg_col,
            "benc": benc_col,
            "wta": wta_r,
            "bta": np.full((NB, 1), float(b_ta[0]), f32),
            "lens": lens[sl].astype(f32).reshape(NB, 1),
            "iota": iota,
            "identb": identb,
        })
    return in_maps


def kernel(**inputs):
    nc = _get_nc()
    in_maps = make_in_maps(inputs)
    res = run_bass_kernel_spmd(nc, in_maps, list(range(NCORES)))
    c = np.concatenate([res.results[i]["c_out"] for i in range(NCORES)], axis=0)
    w = np.concatenate([res.results[i]["w_out"] for i in range(NCORES)], axis=0)
    p = np.concatenate([res.results[i]["p_out"] for i in range(NCORES)], axis=0)
    return c, w, p


# revision 12
# speedup vs baseline: 1.4690x; 1.2455x over previous
"""AttForwardTA (location-aware attention + transition agent) on 8 TRN2 NeuronCores.

Data-parallel: batch 64 is split 8 ways; each core runs the identical program on
its 8 batch rows.  The big matmuls (pre-projection of enc, context) run in
float32r (TF32-like, 1 PE cycle/row); the small conv/decoder/transition-agent
matmuls run in bf16; softmax / prior / renormalization stay in exact f32.
Batches are processed in two groups of four so softmax DVE work overlaps the
next group's PE work and the encoder stash fits SBUF.
"""
import sys

sys.path.insert(0, "/opt/trn_rl_repo")
import numpy as np
import ml_dtypes
from contextlib import ExitStack

import concourse.bass as bass
import concourse.tile as tile
from concourse import mybir, bacc
from concourse.bass_utils import run_bass_kernel_spmd

F32 = mybir.dt.float32
F32R = mybir.dt.float32r
BF16 = mybir.dt.bfloat16
AF = mybir.ActivationFunctionType
ALU = mybir.AluOpType

NB = 8          # batches per core
GRP = 2         # batch groups
GB = NB // GRP  # batches per group
NCORES = 8
T = 1000
E = 512
D = 1024
A = 128
CH = 32
KW = 31
ODIM = 80
TT = 8          # T tiles of 128 (last is 104)
TLEN = [128] * 7 + [104]
EC = E // 128
DC = D // 128
HALVES = [(0, 512), (512, 488)]   # psum-bank-aligned split of T


def _emit(nc):
    enc_h = nc.dram_tensor("enc", [NB, T, E], F32, kind="ExternalInput")
    ap_h = nc.dram_tensor("att_prev", [NB, T], F32, kind="ExternalInput")
    dz_h = nc.dram_tensor("dec_z", [NB, D], F32, kind="ExternalInput")
    op_h = nc.dram_tensor("out_prev", [NB, ODIM], F32, kind="ExternalInput")
    wenc_h = nc.dram_tensor("w_enc", [E, A], F32, kind="ExternalInput")
    wdec_h = nc.dram_tensor("w_dec", [D, A], F32, kind="ExternalInput")
    watt_h = nc.dram_tensor("w_att", [CH, A], F32, kind="ExternalInput")
    conv_h = nc.dram_tensor("conv2", [CH, KW], F32, kind="ExternalInput")
    wg_h = nc.dram_tensor("wg", [A, 1], F32, kind="ExternalInput")
    benc_h = nc.dram_tensor("benc", [A, 1], F32, kind="ExternalInput")
    wta_h = nc.dram_tensor("wta", [13 * 128, 1], F32, kind="ExternalInput")
    bta_h = nc.dram_tensor("bta", [NB, 1], F32, kind="ExternalInput")
    len_h = nc.dram_tensor("lens", [NB, 1], F32, kind="ExternalInput")
    iota_h = nc.dram_tensor("iota", [1, T], F32, kind="ExternalInput")
    idb_h = nc.dram_tensor("identb", [128, 128], BF16, kind="ExternalInput")

    c_out = nc.dram_tensor("c_out", [NB, E], F32, kind="ExternalOutput")
    w_out = nc.dram_tensor("w_out", [NB, T], F32, kind="ExternalOutput")
    p_out = nc.dram_tensor("p_out", [NB, 1], F32, kind="ExternalOutput")

    with tile.TileContext(nc) as tc:
        with ExitStack() as ctx:
            const = ctx.enter_context(tc.tile_pool(name="const", bufs=1))
            ph2 = ctx.enter_context(tc.tile_pool(name="ph2", bufs=1))
            encp = ctx.enter_context(tc.tile_pool(name="encp", bufs=8))
            encTp = ctx.enter_context(tc.tile_pool(name="encTp", bufs=4))
            wrk = ctx.enter_context(tc.tile_pool(name="wrk", bufs=2))
            ppt = ctx.enter_context(tc.tile_pool(name="ppt", bufs=2, space="PSUM"))
            ppx = ctx.enter_context(tc.tile_pool(name="ppx", bufs=2, space="PSUM"))
            pps = ctx.enter_context(tc.tile_pool(name="pps", bufs=2, space="PSUM"))

            def cload(name, shape, dt, src, eng=None):
                t = const.tile(shape, dt, tag=name)
                (eng or nc.gpsimd).dma_start(out=t[:], in_=src)
                return t

            # ---- constants / small inputs ----
            identb = cload("identb", [128, 128], BF16, idb_h.ap(), nc.sync)
            # w_enc chunks side by side: wenc_sb[p, k*128+a] = w_enc[k*128+p, a]
            wenc_sb = const.tile([128, E], BF16, tag="wenc")
            nc.gpsimd.dma_start(
                out=wenc_sb[:],
                in_=bass.AP(tensor=wenc_h.ap().tensor, offset=0,
                            ap=[[A, 128], [128 * A, EC], [1, A]]),
            )
            wdec_sb = const.tile([128, D], BF16, tag="wdec")
            nc.gpsimd.dma_start(
                out=wdec_sb[:],
                in_=bass.AP(tensor=wdec_h.ap().tensor, offset=0,
                            ap=[[A, 128], [128 * A, DC], [1, A]]),
            )
            watt_sb = cload("watt", [CH, A], BF16, watt_h.ap())
            conv_sb = cload("conv", [CH, KW], BF16, conv_h.ap())
            wg_sb = cload("wg", [A, 1], BF16, wg_h.ap())
            benc_sb = cload("benc", [A, 1], F32, benc_h.ap(), nc.sync)
            wta_sb = const.tile([128, 13], BF16, tag="wta")
            nc.gpsimd.dma_start(
                out=wta_sb[:],
                in_=bass.AP(tensor=wta_h.ap().tensor, offset=0,
                            ap=[[1, 128], [128, 13]]),
            )
            bta_sb = cload("bta", [NB, 1], F32, bta_h.ap(), nc.sync)
            dzbf = cload("dzbf", [NB, D], BF16, dz_h.ap())
            opbf = cload("opbf", [NB, ODIM], BF16, op_h.ap())
            # padded att_prev (bf16) for the location conv im2col
            pad_sb = const.tile([NB, T + 30], BF16, tag="pad")
            nc.vector.memset(pad_sb[:], 0.0)
            nc.gpsimd.dma_start(out=pad_sb[:, 15:15 + T], in_=ap_h.ap())

            iota_sb = const.tile([GB, T], F32, tag="iota")
            nc.sync.dma_start(
                out=iota_sb[:],
                in_=bass.AP(tensor=iota_h.ap().tensor, offset=0, ap=[[0, GB], [1, T]]),
            )
            ap32_g = []
            negm_g = []
            for g in range(GRP):
                apg = const.tile([GB, T], F32, tag=f"ap32_{g}")
                nc.sync.dma_start(out=apg[:], in_=ap_h.ap()[g * GB:(g + 1) * GB, :])
                ap32_g.append(apg)
                leng = const.tile([GB, 1], F32, tag=f"len_{g}")
                nc.sync.dma_start(out=leng[:], in_=len_h.ap()[g * GB:(g + 1) * GB, :])
                nm = const.tile([GB, T], F32, tag=f"negm_{g}")
                nc.vector.tensor_scalar(
                    out=nm[:], in0=iota_sb[:],
                    scalar1=leng[:, 0:1], scalar2=-1.0e30,
                    op0=ALU.is_ge, op1=ALU.mult,
                )
                negm_g.append(nm)

            # ---- dec_z^T (reused by the transition-agent tail) and dec_t ----
            dzT = const.tile([128, DC * NB], BF16, tag="dzT")
            for d in range(DC):
                tr = pps.tile([128, NB], BF16, tag="ps")
                nc.tensor.transpose(
                    out=tr[:], in_=dzbf[:, d * 128:(d + 1) * 128],
                    identity=identb[0:NB, 0:NB],
                )
                nc.vector.tensor_copy(out=dzT[:, d * NB:(d + 1) * NB], in_=tr[:])
            pdec = pps.tile([128, NB], F32, tag="ps")
            for d in range(DC):
                nc.tensor.matmul(
                    pdec[:], wdec_sb[:, d * 128:(d + 1) * 128],
                    dzT[:, d * NB:(d + 1) * NB],
                    start=(d == 0), stop=(d == DC - 1),
                )
            bias_sb = const.tile([128, NB], F32, tag="bias")
            nc.vector.tensor_scalar_add(bias_sb[:], pdec[:], benc_sb[:, 0:1])

            # ---- folded conv weights: g[k, a] = sum_c conv[c, k] * w_att[c, a] ----
            pg = pps.tile([KW, A], F32, tag="ps")
            nc.tensor.matmul(pg[:], conv_sb[:], watt_sb[:], start=True, stop=True)
            g_sb = const.tile([KW, A], BF16, tag="g")
            nc.vector.tensor_copy(out=g_sb[:], in_=pg[:])

            c_all = const.tile([NB, E], F32, tag="c_all")
            enc_tiles = [None] * NB
            e_alls = []
            for g in range(GRP):
                ea = ph2.tile([GB, T], F32, tag=f"e_all_{g}")
                e_alls.append(ea)

            def phase1(b):
                g = b // GB
                e_all = e_alls[g]
                if True:
                    enc_b = encp.tile([128, TT * E], BF16, tag="enc")
                    enc_tiles[b] = enc_b
                    nc.gpsimd.dma_start(
                        out=enc_b[:, 0:7 * E],
                        in_=enc_h.ap()[b, 0:896, :].rearrange(
                            "(t p) e -> p t e", p=128),
                    )
                    nc.gpsimd.dma_start(
                        out=enc_b[0:104, 7 * E:8 * E],
                        in_=enc_h.ap()[b, 896:T, :],
                    )

                    encTs = []
                    for k in range(EC):
                        ek = encTp.tile([128, T], BF16, tag="encT")
                        ptk = ppt.tile([128, 1024], BF16, tag="pt")
                        for t in range(TT):
                            tl = TLEN[t]
                            nc.tensor.transpose(
                                out=ptk[:, t * 128:t * 128 + tl],
                                in_=enc_b[0:tl,
                                          t * E + k * 128:t * E + (k + 1) * 128],
                                identity=identb[0:tl, 0:tl],
                            )
                        if k % 2 == 0:
                            nc.vector.tensor_copy(out=ek[:], in_=ptk[:, 0:T])
                        else:
                            nc.scalar.copy(out=ek[:], in_=ptk[:, 0:T])
                        encTs.append(ek)

                    X_b = wrk.tile([KW, T], BF16, tag="X")
                    pad_ap = pad_sb[:]
                    nc.gpsimd.dma_start(
                        out=X_b[:],
                        in_=bass.AP(
                            tensor=pad_ap.tensor,
                            offset=pad_ap.offset + b * (T + 30),
                            ap=[[T + 30, 1], [1, KW], [1, T]],
                        ),
                    )

                    px = ppx.tile([128, T], F32, tag="px")
                    for h0, hw_ in HALVES:
                        for k in range(EC):
                            nc.tensor.matmul(
                                px[:, h0:h0 + hw_],
                                wenc_sb[:, k * 128:(k + 1) * 128],
                                encTs[k][:, h0:h0 + hw_],
                                start=(k == 0), stop=False,
                            )
                        nc.tensor.matmul(
                            px[:, h0:h0 + hw_], g_sb[:], X_b[:, h0:h0 + hw_],
                            start=False, stop=True,
                        )

                    tanh_b = wrk.tile([128, T], BF16, tag="tanh")
                    nc.scalar.activation(
                        tanh_b[:], px[:], AF.Tanh, bias=bias_sb[:, b:b + 1],
                        scale=1.0,
                    )

                    erow = wrk.tile([1, T], F32, tag="erow")
                    for hi, (h0, hw_) in enumerate(HALVES):
                        pe = pps.tile([1, 512], F32, tag="ps")
                        nc.tensor.matmul(
                            pe[0:1, 0:hw_], wg_sb[:], tanh_b[:, h0:h0 + hw_],
                            start=True, stop=True,
                        )
                        if (b + hi) % 2 == 0:
                            nc.vector.tensor_copy(
                                out=erow[0:1, h0:h0 + hw_], in_=pe[0:1, 0:hw_])
                        else:
                            nc.scalar.copy(
                                out=erow[0:1, h0:h0 + hw_], in_=pe[0:1, 0:hw_])
                    nc.sync.dma_start(
                        out=e_all[b - g * GB:b - g * GB + 1, :], in_=erow[:])

            def phase23(g):
                e_all = e_alls[g]
                # ============ phase 2: softmax + prior + renormalize ==========
                e_m = ph2.tile([GB, T], F32, tag="e_m")
                nc.vector.tensor_add(e_m[:], e_all[:], negm_g[g][:])
                rmax = ph2.tile([GB, 1], F32, tag="rmax")
                nc.vector.tensor_reduce(
                    out=rmax[:], in_=e_m[:], axis=mybir.AxisListType.X, op=ALU.max)
                rneg = ph2.tile([GB, 1], F32, tag="rneg")
                nc.vector.tensor_scalar_mul(rneg[:], rmax[:], -1.0)
                ex_t = ph2.tile([GB, T], F32, tag="ex_t")
                ssum = ph2.tile([GB, 1], F32, tag="ssum")
                nc.scalar.activation(
                    ex_t[:], e_m[:], AF.Exp, bias=rneg[:, 0:1], scale=1.0,
                    accum_out=ssum[:],
                )
                thr = ph2.tile([GB, 1], F32, tag="thr")
                nc.vector.tensor_scalar_mul(thr[:], ssum[:], 2.0e-6)
                padd = ph2.tile([GB, T], F32, tag="padd")
                nc.vector.tensor_add(
                    padd[:, 1:T], ap32_g[g][:, 1:T], ap32_g[g][:, 0:T - 1])
                nc.vector.tensor_copy(out=padd[:, 0:1], in_=ap32_g[g][:, 0:1])
                q_t = ph2.tile([GB, T], F32, tag="q_t")
                nc.vector.tensor_mul(q_t[:], padd[:], ex_t[:])
                qc = ph2.tile([GB, T], F32, tag="qc")
                qsum = ph2.tile([GB, 1], F32, tag="qsum")
                nc.vector.tensor_scalar(
                    out=qc[:], in0=q_t[:], scalar1=thr[:, 0:1], scalar2=None,
                    op0=ALU.max, op1=ALU.add, accum_out=qsum[:],
                )
                winv = ph2.tile([GB, 1], F32, tag="winv")
                nc.vector.reciprocal(winv[:], qsum[:])
                w32 = ph2.tile([GB, T], F32, tag="w32")
                nc.vector.tensor_scalar_mul(w32[:], qc[:], winv[:, 0:1])
                nc.sync.dma_start(
                    out=w_out.ap()[g * GB:(g + 1) * GB, :], in_=w32[:])

                wr = ph2.tile([GB, T], BF16, tag="wr")
                nc.vector.tensor_copy(out=wr[:], in_=w32[:])
                wT_sb = ph2.tile([128, TT * GB], BF16, tag="wT")
                for j in range(TT):
                    tl = TLEN[j]
                    tr = pps.tile([128, GB], BF16, tag="ps")
                    nc.tensor.transpose(
                        out=tr[0:tl, :], in_=wr[:, j * 128:j * 128 + tl],
                        identity=identb[0:GB, 0:GB],
                    )
                    nc.vector.tensor_copy(
                        out=wT_sb[0:tl, j * GB:(j + 1) * GB], in_=tr[0:tl, :])

                # ============ phase 3: context ============
                for b in range(g * GB, (g + 1) * GB):
                    pc = pps.tile([1, 512], F32, tag="ps")
                    for j in range(TT):
                        tl = TLEN[j]
                        nc.tensor.matmul(
                            pc[:],
                            wT_sb[0:tl, j * GB + (b - g * GB):
                                  j * GB + (b - g * GB) + 1],
                            enc_tiles[b][0:tl, j * E:(j + 1) * E],
                            start=(j == 0), stop=(j == TT - 1),
                        )
                    crow = wrk.tile([1, E], F32, tag="crow")
                    if b % 2 == 0:
                        nc.vector.tensor_copy(out=crow[:], in_=pc[:])
                    else:
                        nc.scalar.copy(out=crow[:], in_=pc[:])
                    nc.sync.dma_start(out=c_all[b:b + 1, :], in_=crow[:])

            for b in range(NB):
                phase1(b)
                if b == GB + 1:
                    phase23(0)
            phase23(1)

            nc.sync.dma_start(out=c_out.ap(), in_=c_all[:])

            # ================= phase 4: transition agent prob =================
            cbf = const.tile([NB, E], BF16, tag="cbf")
            nc.vector.tensor_copy(out=cbf[:], in_=c_all[:])
            cT = const.tile([128, 4 * NB], BF16, tag="cT")
            for i in range(4):
                tr = pps.tile([128, NB], BF16, tag="ps")
                nc.tensor.transpose(
                    out=tr[:], in_=cbf[:, i * 128:(i + 1) * 128],
                    identity=identb[0:NB, 0:NB],
                )
                nc.vector.tensor_copy(out=cT[:, i * NB:(i + 1) * NB], in_=tr[:])
            opT = const.tile([128, NB], BF16, tag="opT")
            trop = pps.tile([128, NB], BF16, tag="ps")
            nc.tensor.transpose(
                out=trop[0:ODIM, :], in_=opbf[:, 0:ODIM],
                identity=identb[0:NB, 0:NB],
            )
            nc.vector.tensor_copy(out=opT[0:ODIM, :], in_=trop[0:ODIM, :])

            ps_s = pps.tile([NB, 1], F32, tag="ps")
            for i in range(13):
                if i < 4:
                    lhsT = cT[:, i * NB:(i + 1) * NB]
                    rhs = wta_sb[:, i:i + 1]
                elif i < 12:
                    lhsT = dzT[:, (i - 4) * NB:(i - 3) * NB]
                    rhs = wta_sb[:, i:i + 1]
                else:
                    lhsT = opT[0:ODIM, :]
                    rhs = wta_sb[0:ODIM, i:i + 1]
                nc.tensor.matmul(ps_s[:], lhsT, rhs, start=(i == 0), stop=(i == 12))
            s_sb = const.tile([NB, 1], F32, tag="s_sb")
            nc.vector.tensor_scalar_add(s_sb[:], ps_s[:], bta_sb[:, 0:1])
            exs = const.tile([NB, 1], F32, tag="exs")
            nc.scalar.activation(exs[:], s_sb[:], AF.Exp, bias=0.0, scale=-1.0)
            p1 = const.tile([NB, 1], F32, tag="p1")
            nc.vector.tensor_scalar_add(p1[:], exs[:], 1.0)
            prob = const.tile([NB, 1], F32, tag="prob")
            nc.vector.reciprocal(prob[:], p1[:])
            nc.sync.dma_start(out=p_out.ap(), in_=prob[:])

    return nc


_NC = None


def _get_nc():
    global _NC
    if _NC is None:
        nc = bacc.Bacc("TRN2", target_bir_lowering=False, debug=False,
                       num_devices=NCORES)
        _emit(nc)
        nc.compile()
        _NC = nc
    return _NC


def make_in_maps(inputs):
    f32 = np.float32
    enc = np.asarray(inputs["enc_hs_pad"], f32)
    lens = np.asarray(inputs["enc_hs_len"], np.int32)
    dec_z = np.asarray(inputs["dec_z"], f32)
    att_prev = np.asarray(inputs["att_prev"], f32)
    out_prev = np.asarray(inputs["out_prev"], f32)
    w_enc = np.asarray(inputs["w_enc"], f32)
    w_dec = np.asarray(inputs["w_dec"], f32)
    w_att = np.asarray(inputs["w_att"], f32)
    conv_w = np.asarray(inputs["conv_w"], f32)
    w_g = np.asarray(inputs["w_g"], f32)
    w_ta = np.asarray(inputs["w_ta"], f32)
    b_ta = np.asarray(inputs["b_ta"], f32)
    b_enc = np.asarray(inputs["b_enc"], f32)

    conv2 = np.ascontiguousarray(conv_w[:, 0, :])                  # [32, 31]
    wg_col = np.ascontiguousarray(w_g.reshape(A, 1))
    benc_col = np.ascontiguousarray(b_enc.reshape(A, 1))
    # ta input order on device: [c | dec_z | out_prev]; pad to 13*128 rows
    wta_r = np.zeros((13 * 128, 1), f32)
    wta_r[0:E, 0] = w_ta[0:E, 0]
    wta_r[E:E + D, 0] = w_ta[E + ODIM:E + ODIM + D, 0]
    wta_r[E + D:E + D + ODIM, 0] = w_ta[E:E + ODIM, 0]
    iota = np.arange(T, dtype=f32).reshape(1, T)
    identb = np.eye(128, dtype=f32).astype(ml_dtypes.bfloat16)

    in_maps = []
    for i in range(NCORES):
        sl = slice(i * NB, (i + 1) * NB)
        in_maps.append({
            "enc": np.ascontiguousarray(enc[sl]),
            "att_prev": np.ascontiguousarray(att_prev[sl]),
            "dec_z": np.ascontiguousarray(dec_z[sl]),
            "out_prev": np.ascontiguousarray(out_prev[sl]),
            "w_enc": w_enc,
            "w_dec": w_dec,
            "w_att": w_att,
            "conv2": conv2,
            "wg": wg_col,
            "benc": benc_col,
            "wta": wta_r,
            "bta": np.full((NB, 1), float(b_ta[0]), f32),
            "lens": lens[sl].astype(f32).reshape(NB, 1),
            "iota": iota,
            "identb": identb,
        })
    return in_maps


def kernel(**inputs):
    nc = _get_nc()
    in_maps = make_in_maps(inputs)
    res = run_bass_kernel_spmd(nc, in_maps, list(range(NCORES)))
    c = np.concatenate([res.results[i]["c_out"] for i in range(NCORES)], axis=0)
    w = np.concatenate([res.results[i]["w_out"] for i in range(NCORES)], axis=0)
    p = np.concatenate([res.results[i]["p_out"] for i in range(NCORES)], axis=0)
    return c, w, p
